# revision 16
# baseline (speedup 1.0000x reference)
"""Trainium2 Bass kernel for the RNODE + MLP readout model.

Math: x_{t+1} = a*x_t + dt*W@sigmoid(x_t+theta) + dt*in_mat@u_t, a = 1-dt/tau
      out_t  = fc2 @ relu(fc1 @ x_{t+1} + b1) + b2
      returns (outputs [3,T], potentials [3,T]), T = 500000.

Strategy: 8-way time-chunk parallelism (62500 steps/core + 2012-step burn-in;
the dynamics contract at a=0.99/step so the chunk-initial state is forgotten
within the burn-in), and within a core the sequential recurrence is solved by
Picard iteration: 6 sweeps of [sigmoid -> blockdiag matmul -> linear IIR],
where the IIR (constant coefficient a) runs as a hardware tensor_tensor_scan
over a 42-block x 1536-step slab layout plus a tiny cross-block combine scan.
The MLP readout is a batched PE matmul pass over all timesteps.
"""

import numpy as np

import concourse.bass as bass
import concourse.tile as tile
from concourse import mybir
from concourse.bass_utils import run_bass_kernel_spmd

DT = 0.01
NB = 42            # blocks per core (3*NB = 126 partitions)
S = 1536           # steps per block
TEXT = NB * S      # 64512 extended steps per core
TCHUNK = 62500     # owned steps per core
K = TEXT - TCHUNK  # 2012 burn-in steps
NSWEEP = 6
P = 3 * NB         # 126
NCORES = 8
F32 = mybir.dt.float32
LINEARIZE = False


def _legalize_waits(nc, maxw=1):
    """This walrus build encodes at most one sync-wait per instruction;
    move excess on_wait entries into standalone EventSemaphore waits."""
    for fn in nc.m.functions:
        for blk in fn.blocks:
            out = []
            for inst in blk.instructions:
                si = inst.sync_info
                if si is not None and len(si.on_wait) > maxw:
                    waits = list(si.on_wait)
                    excess, keep = waits[:-maxw], waits[-maxw:]
                    for w in excess:
                        ev = mybir.InstEventSemaphore(
                            name=nc.get_next_instruction_name(), ins=[], outs=[]
                        )
                        ev.engine = inst.engine
                        ev.sync_info = mybir.SyncInfo(on_wait=[w], on_update=[])
                        out.append(ev)
                    si.on_wait = keep
                out.append(inst)
            blk.instructions = out

_cache = {}


def _build_program(a: float):
    aS = float(np.float64(a) ** S)
    nc = bass.Bass()

    u_ext = nc.dram_tensor("u_ext", [3, TEXT], F32, kind="ExternalInput")
    w_blk = nc.dram_tensor("w_blk", [P, P], F32, kind="ExternalInput")
    in_blk = nc.dram_tensor("in_blk", [P, P], F32, kind="ExternalInput")
    theta_p = nc.dram_tensor("theta_p", [P, 1], F32, kind="ExternalInput")
    ident = nc.dram_tensor("ident", [P, P], F32, kind="ExternalInput")
    fc1t2 = nc.dram_tensor("fc1t2", [6, 128], F32, kind="ExternalInput")
    b1col = nc.dram_tensor("b1col", [128, 1], F32, kind="ExternalInput")
    fc2t2 = nc.dram_tensor("fc2t2", [128, 6], F32, kind="ExternalInput")
    b2col = nc.dram_tensor("b2col", [6, 1], F32, kind="ExternalInput")

    pot = nc.dram_tensor("pot", [3, TEXT], F32, kind="ExternalOutput")
    mlp = nc.dram_tensor("mlp", [3, TEXT], F32, kind="ExternalOutput")

    Sig = mybir.ActivationFunctionType.Sigmoid
    Relu = mybir.ActivationFunctionType.Relu
    MUL = mybir.AluOpType.mult
    ADD = mybir.AluOpType.add

    with tile.TileContext(nc, linearize=LINEARIZE) as tc:
        with (
            tc.tile_pool(name="const", bufs=1) as const,
            tc.tile_pool(name="work", bufs=1) as work,
        ):
            # ---- constants to SBUF ----
            u_slab = const.tile([P, S], F32)
            for c in range(3):
                nc.sync.dma_start(
                    out=u_slab.rearrange("(b c) s -> c b s", c=3)[c],
                    in_=u_ext[c, :].rearrange("(b s) -> b s", b=NB),
                )
            wb = const.tile([P, P], F32)
            nc.sync.dma_start(out=wb, in_=w_blk[:, :])
            ib = const.tile([P, P], F32)
            nc.sync.dma_start(out=ib, in_=in_blk[:, :])
            th = const.tile([P, 1], F32)
            nc.sync.dma_start(out=th, in_=theta_p[:, :])
            idt = const.tile([P, P], F32)
            nc.sync.dma_start(out=idt, in_=ident[:, :])
            f1 = const.tile([6, 128], F32)
            nc.sync.dma_start(out=f1, in_=fc1t2[:, :])
            b1 = const.tile([128, 1], F32)
            nc.sync.dma_start(out=b1, in_=b1col[:, :])
            f2 = const.tile([128, 6], F32)
            nc.sync.dma_start(out=f2, in_=fc2t2[:, :])
            b2 = const.tile([6, 1], F32)
            nc.sync.dma_start(out=b2, in_=b2col[:, :])

            a_tile = const.tile([P, S], F32)
            nc.vector.memset(a_tile, a)
            aS_row = const.tile([1, NB], F32)
            nc.vector.memset(aS_row, aS)
            one11 = const.tile([1, 1], F32)
            nc.vector.memset(one11, 1.0)

            x_t = work.tile([P, S], F32)      # trajectory (potentials slab)
            x1_t = work.tile([P, S], F32)     # scan1 result
            sig = work.tile([P, S], F32)
            irow = work.tile([1, P], F32)
            nc.vector.memset(x_t, 0.0)

            irow_v = irow.rearrange("p (b c) -> p b c", c=3)

            # ---- Picard sweeps ----
            with (
                tc.tile_pool(name="gp", bufs=1, space="PSUM") as gp,
                tc.tile_pool(name="tinyp", bufs=2, space="PSUM") as tinyp,
            ):
                # block-initial values of the current trajectory x_t; zero at
                # start, afterwards the combine's initcol (which equals the
                # previous block's scan2 final — the shifted col-0 value).
                icprev = tinyp.tile([P, 1], F32, tag="initcol")
                nc.vector.memset(icprev, 0.0)
                for _k in range(NSWEEP):
                    # shifted sigmoid: sig[:,1:] = sigmoid(x[:,:-1]+theta)
                    nc.scalar.activation(
                        out=sig[:, 1:S],
                        in_=x_t[:, 0 : S - 1],
                        func=Sig,
                        bias=th[:, 0:1],
                    )
                    # col 0: sigmoid of previous block's final = icprev
                    nc.scalar.activation(
                        out=sig[:, 0:1], in_=icprev, func=Sig, bias=th[:, 0:1]
                    )

                    # G = w_blk^T @ sig + in_blk^T @ u   (PSUM, 3 chunks of 512)
                    G = gp.tile([P, S], F32, tag="G")
                    for j in range(3):
                        cs = slice(j * 512, (j + 1) * 512)
                        nc.tensor.matmul(
                            G[:, cs], lhsT=wb, rhs=sig[:, cs], start=True, stop=False
                        )
                        nc.tensor.matmul(
                            G[:, cs], lhsT=ib, rhs=u_slab[:, cs], start=False, stop=True
                        )

                    # scan1: within-block IIR from 0
                    nc.vector.tensor_tensor_scan(
                        out=x1_t, data0=a_tile, data1=G, initial=0.0, op0=MUL, op1=ADD
                    )

                    # combine block finals into per-block initial values
                    frow = tinyp.tile([1, P], F32, tag="frow")
                    nc.tensor.matmul(
                        frow, lhsT=x1_t[:, S - 1 : S], rhs=idt, start=True, stop=True
                    )
                    nc.vector.memset(irow[:, 0:3], 0.0)
                    frow_v = frow.rearrange("p (b c) -> p b c", c=3)
                    for c in range(3):
                        nc.vector.tensor_tensor_scan(
                            out=irow_v[:, 1:NB, c],
                            data0=aS_row[:, 0 : NB - 1],
                            data1=frow_v[:, 0 : NB - 1, c],
                            initial=0.0,
                            op0=MUL,
                            op1=ADD,
                        )
                    initcol = tinyp.tile([P, 1], F32, tag="initcol")
                    nc.tensor.matmul(
                        initcol, lhsT=irow, rhs=one11, start=True, stop=True
                    )

                    # scan2: full IIR with correct block-initial values
                    nc.vector.tensor_tensor_scan(
                        out=x_t,
                        data0=a_tile,
                        data1=G,
                        initial=initcol[:, 0:1],
                        op0=MUL,
                        op1=ADD,
                    )
                    icprev = initcol

            # ---- potentials out ----
            for c in range(3):
                nc.sync.dma_start(
                    out=pot[c, :].rearrange("(b s) -> b s", b=NB),
                    in_=x_t.rearrange("(b c) s -> c b s", c=3)[c],
                )

            # ---- repack to [6, TEXT/2] for the MLP (2 halves of 21 blocks) ----
            x2 = work.tile([6, TEXT // 2], F32)
            for h in range(2):
                for c in range(3):
                    nc.sync.dma_start(
                        out=x2[3 * h + c : 3 * h + c + 1, :],
                        in_=x_t.rearrange("(b c) s -> c b s", c=3)[c][
                            21 * h : 21 * h + 21, :
                        ],
                    )

            # ---- MLP: out = fc2 @ relu(fc1 @ x + b1) + b2, 2 time-groups ----
            NCH = (TEXT // 2) // 512  # 63
            with (
                tc.tile_pool(name="mwork", bufs=3) as mwork,
                tc.tile_pool(name="mpsum", bufs=2, space="PSUM") as mpsum,
            ):
                for ch in range(NCH):
                    cs = slice(ch * 512, (ch + 1) * 512)
                    hp = mpsum.tile([128, 512], F32, tag="hp")
                    nc.tensor.matmul(hp, lhsT=f1, rhs=x2[:, cs], start=True, stop=True)
                    hr = mwork.tile([128, 512], F32, tag="hr")
                    nc.scalar.activation(out=hr, in_=hp, func=Relu, bias=b1[:, 0:1])
                    op_ = mpsum.tile([6, 512], F32, tag="op")
                    nc.tensor.matmul(op_, lhsT=f2, rhs=hr, start=True, stop=True)
                    ot = mwork.tile([6, 512], F32, tag="ot")
                    nc.vector.tensor_scalar(ot, op_, b2[:, 0:1], None, ADD)
                    for h in range(2):
                        off = h * (TEXT // 2) + ch * 512
                        nc.sync.dma_start(
                            out=mlp[:, off : off + 512],
                            in_=ot[3 * h : 3 * h + 3, :],
                        )

    _legalize_waits(nc, 1)
    return nc


def _prep_inputs(u, in_mat, W, theta, fc1_w, fc1_b, fc2_w, fc2_b):
    u = np.ascontiguousarray(np.asarray(u, np.float32))
    W = np.asarray(W, np.float32)
    in_mat = np.asarray(in_mat, np.float32)
    theta = np.asarray(theta, np.float32)
    fc1_w = np.asarray(fc1_w, np.float32)
    fc1_b = np.asarray(fc1_b, np.float32)
    fc2_w = np.asarray(fc2_w, np.float32)
    fc2_b = np.asarray(fc2_b, np.float32)

    dtf = np.float32(DT)
    w_blk = np.zeros((P, P), np.float32)
    in_blk = np.zeros((P, P), np.float32)
    wT = (dtf * W).T.astype(np.float32)
    iT = (dtf * in_mat).T.astype(np.float32)
    for b in range(NB):
        w_blk[3 * b : 3 * b + 3, 3 * b : 3 * b + 3] = wT
        in_blk[3 * b : 3 * b + 3, 3 * b : 3 * b + 3] = iT
    theta_p = np.tile(theta, NB).astype(np.float32).reshape(P, 1)
    ident = np.eye(P, dtype=np.float32)

    fc1t2 = np.zeros((6, 128), np.float32)
    fc1t2[0:3, 0:64] = fc1_w.T
    fc1t2[3:6, 64:128] = fc1_w.T
    b1col = np.concatenate([fc1_b, fc1_b]).astype(np.float32).reshape(128, 1)
    fc2t2 = np.zeros((128, 6), np.float32)
    fc2t2[0:64, 0:3] = fc2_w.T
    fc2t2[64:128, 3:6] = fc2_w.T
    b2col = np.concatenate([fc2_b, fc2_b]).astype(np.float32).reshape(6, 1)

    shared = {
        "w_blk": w_blk,
        "in_blk": in_blk,
        "theta_p": theta_p,
        "ident": ident,
        "fc1t2": fc1t2,
        "b1col": b1col,
        "fc2t2": fc2t2,
        "b2col": b2col,
    }
    in_maps = []
    for core in range(NCORES):
        t0 = 0 if core == 0 else core * TCHUNK - K
        u_ext = np.ascontiguousarray(u[:, t0 : t0 + TEXT])
        in_maps.append({"u_ext": u_ext, **shared})
    return in_maps


def kernel(u, in_mat, W, theta, tau, fc1_w, fc1_b, fc2_w, fc2_b, _trace=False):
    tau_f = float(np.asarray(tau))
    a = float(np.float32(1.0) - np.float32(DT) / np.float32(tau_f))
    key = ("prog", round(a, 12))
    if key not in _cache:
        _cache[key] = _build_program(a)
    nc = _cache[key]

    in_maps = _prep_inputs(u, in_mat, W, theta, fc1_w, fc1_b, fc2_w, fc2_b)
    res = run_bass_kernel_spmd(nc, in_maps, core_ids=list(range(NCORES)), trace=_trace)

    outputs = np.zeros((3, 500000), np.float32)
    potentials = np.zeros((3, 500000), np.float32)
    for core in range(NCORES):
        off = 0 if core == 0 else K
        sl = slice(core * TCHUNK, (core + 1) * TCHUNK)
        outputs[:, sl] = res.results[core]["mlp"][:, off : off + TCHUNK]
        potentials[:, sl] = res.results[core]["pot"][:, off : off + TCHUNK]
    kernel.last_result = res
    return outputs, potentials
